# revision 13
# baseline (speedup 1.0000x reference)
"""Trainium2 Bass kernel for nn_Block_16174846837078 (moe_routing).

Data-parallel over batch: each of the 8 cores gets 4 "large"-half and 4
"small"-half samples. All compute runs on-device in a single NEFF.

v2 layout/pipelining rework over the v1 baseline:
  - No DRAM round-trips: xn2T (post-LN2 transposed activations) and x2
    (post-attention residual) stay resident in SBUF through the MLP.
  - Attention core: per (sample, head) the three key-chunk score matmuls
    land in one 3-bank PSUM tile; ONE batched Exp activation evicts all
    three; the PV accumulation reuses bank 0 of the same tile.  Softmax
    denominators ride as a ones-column in PV; 1/den via DVE reciprocal;
    the per-head normalization is a GpSimd partition-broadcast plus a
    GpSimd elementwise multiply (no TensorE broadcast matmuls).
  - q/k generation for pair p+1 is issued alongside the attention core of
    pair p, giving the list scheduler dense PE work to hide Exp latency.
  - MLP small-half: the three gelu prefix snapshots come from three
    separate PSUM accumulation regions (A=c0:256, B=c256:384, C=c384:768),
    so the fc1 accumulation never pauses for ScalarE reads; gumbel
    combines run on GpSimd.
  - fc2 outputs accumulate in a single 5-bank PSUM tile per token group
    (3x512 + 3x256 packed into banks 3/4).
"""

import numpy as np

P = 128
H = 12
HD = 64
C = 768
HID = 3072
N = 257
SL = 4              # large samples per core
SS = 4              # small samples per core
T = SL * N          # 1028 tokens per half per core
NCORES = 8
EPS = 1e-5

# token chunks of 128 over one half
TCH = [(o, min(P, T - o)) for o in range(0, T, P)]          # 8x128 + 1x4
# key chunks within one sample
KCH = [(0, 128), (128, 128), (256, 1)]
# fp32r matmuls need an even moving-operand free size; pad each sample's
# 257 tokens to 258 in the attention-side (transposed) layouts.
NP = N + 1          # 258
QTL = SL * NP       # 1032
# mlp token groups: (group offset, [(rel off, size), ...])
GRPS = [
    (0, [(0, 128), (128, 128), (256, 128)]),
    (384, [(0, 128), (128, 128), (256, 128)]),
    (768, [(0, 128), (128, 128), (256, 4)]),
]

_CACHE = {}


def _build(taps=False):
    import concourse.bacc as bacc
    import concourse.tile as tile
    from concourse import mybir
    from concourse.masks import make_identity
    import concourse.bass as bass

    dt = mybir.dt
    f32 = dt.float32
    f32r = dt.bfloat16  # matmul-operand dtype (bf16: 1 cyc/elem + FWL)
    AF = mybir.ActivationFunctionType
    OP = mybir.AluOpType

    nc = bacc.Bacc("TRN2", target_bir_lowering=False, debug=False)

    # ---------------- I/O ----------------
    x_d = nc.dram_tensor("x", [SL + SS, N, C], f32, kind="ExternalInput").ap()
    gw_d = nc.dram_tensor("gumbel_weights", [3], f32, kind="ExternalInput").ap()
    vecs = {}
    for nm in ["n1l_g", "n1l_b", "n1s_g", "n1s_b", "n2l_g", "n2l_b",
               "n2s_g", "n2s_b", "b_proj", "b_fc1", "b_fc2"]:
        sz = HID if nm == "b_fc1" else C
        vecs[nm] = nc.dram_tensor(nm, [sz], f32, kind="ExternalInput").ap()
    wqkv_d = nc.dram_tensor("w_qkv", [C, 3 * C], f32, kind="ExternalInput").ap()
    wproj_d = nc.dram_tensor("w_proj", [C, C], f32, kind="ExternalInput").ap()
    wfc1_d = nc.dram_tensor("w_fc1", [C, HID], f32, kind="ExternalInput").ap()
    wfc2_d = nc.dram_tensor("w_fc2", [HID, C], f32, kind="ExternalInput").ap()
    out_d = nc.dram_tensor("out", [SL + SS, N, C], f32, kind="ExternalOutput").ap()

    x_flat = x_d.rearrange("b n c -> (b n) c")          # [2T, C]
    out_flat = out_d.rearrange("b n c -> (b n) c")

    tap_list = {}

    def tap(name, ap):
        if not taps:
            return
        t = nc.dram_tensor(name, list(ap.shape), ap.dtype,
                           kind="ExternalOutput").ap()
        nc.sync.dma_start(out=t, in_=ap)
        tap_list[name] = t

    def bcast_row(vec_ap, parts=P):
        # DRAM [n] -> [parts, n] stride-0 partition broadcast
        return bass.AP(tensor=vec_ap.tensor, offset=vec_ap.offset,
                       ap=[[0, parts], list(vec_ap.ap[0])])

    with tile.TileContext(nc) as tc, \
         nc.allow_low_precision(reason="bf16 matmul operands"):
        with tc.tile_pool(name="const", bufs=1) as const:
            ident = const.tile([P, P], f32, tag="ident")
            make_identity(nc, ident)
            ident_b = const.tile([P, P], f32r, tag="ident_b")
            nc.vector.tensor_copy(ident_b, ident)
            eps_t = const.tile([P, 1], f32, tag="eps")
            nc.vector.memset(eps_t, EPS)
            ones32 = const.tile([P, P], f32, tag="ones32")
            nc.vector.memset(ones32, 1.0)
            ones_row = const.tile([1, P], f32r, tag="ones_row")
            nc.vector.tensor_copy(ones_row, ones32[0:1])
            zero24 = const.tile([P, 24], f32, tag="zero24")
            nc.vector.memset(zero24, 0.0)
            gw_b = const.tile([P, 3], f32, tag="gw")
            nc.gpsimd.dma_start(out=gw_b, in_=bcast_row(gw_d))
            gsum = const.tile([P, 2], f32, tag="gsum")  # [g0+g1, g0+g1+g2]
            nc.vector.tensor_add(gsum[:, 0:1], gw_b[:, 0:1], gw_b[:, 1:2])
            nc.vector.tensor_add(gsum[:, 1:2], gsum[:, 0:1], gw_b[:, 2:3])
            g0c = gw_b[:, 0:1]
            g1c = gw_b[:, 1:2]
            g2c = gw_b[:, 2:3]
            g01c = gsum[:, 0:1]
            g012c = gsum[:, 1:2]

            lncols = {}
            for nm in ["n1l_g", "n1l_b", "n1s_g", "n1s_b",
                       "n2l_g", "n2l_b", "n2s_g", "n2s_b"]:
                t = const.tile([P, 6], f32, tag=f"col_{nm}", name=f"col_{nm}")
                nc.sync.dma_start(out=t, in_=vecs[nm].rearrange("(j p) -> p j", p=P))
                lncols[nm] = t
            b1_col = const.tile([P, 24], f32, tag="b1col")
            nc.sync.dma_start(out=b1_col,
                              in_=vecs["b_fc1"].rearrange("(j p) -> p j", p=P))

            def small_scale3(dst, src, ranges_cols, eng=None):
                eng = eng or nc.vector
                for (a, b), colv in ranges_cols:
                    eng.tensor_scalar_mul(dst[:, a:b], src[:, a:b], colv)

            # ---------------- phase helpers ----------------
            def ln1_phase(half, xnT, ln_pool, pst_pool):
                gc = lncols["n1l_g" if half == 0 else "n1s_g"]
                bc = lncols["n1l_b" if half == 0 else "n1s_b"]
                for s in range(SL):
                    for (kof, sz) in KCH:
                        of_c = s * N + kof       # contiguous source tokens
                        of_p = s * NP + kof      # padded destination
                        x_t = ln_pool.tile([P, C], f32, tag="ln_x")
                        nc.sync.dma_start(
                            out=x_t[0:sz],
                            in_=x_flat[half * T + of_c: half * T + of_c + sz])
                        xg = x_t[0:sz].rearrange("p (g d) -> p g d", g=3)
                        stats = ln_pool.tile([P, 3, 6], f32, tag="ln_stats")
                        for i in range(3):
                            nc.vector.bn_stats(out=stats[0:sz, i], in_=xg[:, i])
                        mv = ln_pool.tile([P, 2], f32, tag="ln_mv")
                        nc.vector.bn_aggr(out=mv[0:sz], in_=stats[0:sz])
                        rstd = ln_pool.tile([P, 1], f32, tag="ln_rstd")
                        nc.scalar.activation(rstd[0:sz], mv[0:sz, 1:2], AF.Sqrt,
                                             bias=eps_t[0:sz], scale=1.0)
                        nc.vector.reciprocal(rstd[0:sz], rstd[0:sz])
                        pre = ln_pool.tile([P, C], f32r, tag="ln_pre")
                        nc.vector.tensor_scalar(pre[0:sz], x_t[0:sz],
                                                scalar1=mv[0:sz, 0:1],
                                                scalar2=rstd[0:sz],
                                                op0=OP.subtract, op1=OP.mult)
                        for j in range(6):
                            pst = pst_pool.tile([P, P], f32r, tag="pst")
                            nc.tensor.transpose(pst[:, 0:sz],
                                                pre[0:sz, j * P:(j + 1) * P],
                                                ident_b[0:sz, 0:sz])
                            nc.scalar.activation(
                                xnT[:, j, of_p:of_p + sz], pst[:, 0:sz],
                                AF.Identity,
                                bias=bc[:, j:j + 1], scale=gc[:, j:j + 1])
                # zero the per-sample pad column so q/k pads come out zero
                pads = xnT.rearrange("p j (s n) -> p j s n", n=NP)[:, :, :, N:N + 1]
                nc.vector.tensor_copy(
                    pads,
                    zero24[:, 0:6 * SL].rearrange(
                        "p (j s) -> p j s", j=6).unsqueeze(3))

            # ---------------- attention ----------------
            def v_gen(half, xnT, wv_sb, vnat_pool, pq_pool, vnat):
                for s in range(SL):
                    for kc, (kof, ksz) in enumerate(KCH):
                        vt = vnat_pool.tile([ksz, H, 65], f32r,
                                            tag=f"v_{s}_{kc}",
                                            name=f"v_{s}_{kc}")
                        nc.vector.tensor_copy(
                            vt[:, :, 64:65],
                            ones32[0:ksz, 0:H].unsqueeze(2))
                        vnat[(s, kc)] = vt
                # full 128-key chunks: two output halves per 2-bank psum tile
                for s in range(SL):
                    for kc in range(2):
                        kof, ksz = KCH[kc]
                        tof = s * NP + kof
                        pvv = pq_pool.tile([P, 1024], f32, tag="pq")
                        for ch in range(2):
                            for kk in range(6):
                                nc.tensor.matmul(
                                    pvv[0:ksz, ch * 512:ch * 512 + 384],
                                    lhsT=xnT[:, kk, tof:tof + ksz],
                                    rhs=wv_sb[:, kk, ch * 384:(ch + 1) * 384],
                                    start=(kk == 0), stop=(kk == 5))
                        for ch in range(2):
                            nc.vector.tensor_copy(
                                vnat[(s, kc)][:, ch * 6:(ch + 1) * 6, 0:64],
                                pvv[0:ksz, ch * 512:ch * 512 + 384].rearrange(
                                    "p (h d) -> p h d", h=6))
                # tail key (#256) of all 4 samples in one batched matmul,
                # then tiny SBUF->SBUF DMAs to re-partition rows.
                xnT_s = xnT.rearrange("p j (s n) -> p j s n", n=NP)
                pvt = pq_pool.tile([P, 1024], f32, tag="pq")
                for ch in range(2):
                    for kk in range(6):
                        nc.tensor.matmul(
                            pvt[0:SL, ch * 512:ch * 512 + 384],
                            lhsT=xnT_s[:, kk, :, 256],
                            rhs=wv_sb[:, kk, ch * 384:(ch + 1) * 384],
                            start=(kk == 0), stop=(kk == 5))
                vtail4 = vnat_pool.tile([SL, H, 64], f32r, tag="vtail4")
                for ch in range(2):
                    nc.vector.tensor_copy(
                        vtail4[:, ch * 6:(ch + 1) * 6, :],
                        pvt[0:SL, ch * 512:ch * 512 + 384].rearrange(
                            "p (h d) -> p h d", h=6))
                for s in range(SL):
                    nc.sync.dma_start(out=vnat[(s, 2)][:, :, 0:64],
                                      in_=vtail4[s:s + 1])

            def qk_gen(half, pair, xnT, wqk_sb, qT, kT, pq_pool):
                for dst, cbase in ((qT, pair * P), (kT, C + pair * P)):
                    for sp in range(2):
                        pvt = pq_pool.tile([P, 1024], f32, tag="pq")
                        for j in range(2):
                            s = 2 * sp + j
                            for kk in range(6):
                                nc.tensor.matmul(
                                    pvt[:, j * 512: j * 512 + NP],
                                    lhsT=wqk_sb[:, kk, cbase:cbase + P],
                                    rhs=xnT[:, kk, s * NP:(s + 1) * NP],
                                    start=(kk == 0), stop=(kk == 5))
                        src = pvt.rearrange("p (b x) -> p b x", x=512)[:, :, 0:NP]
                        dsl = dst[:, 2 * sp * NP: (2 * sp + 2) * NP]
                        nc.vector.tensor_copy(
                            dsl.rearrange("p (b x) -> p b x", x=NP), src)

            def attn_pair(half, pair, qT, kT, vnat, oall,
                          pss_pool, e_pool, dg_pool, recb_pool):
                dg = dg_pool.tile([65, 2 * SL, NP], f32, tag="dg")
                osls = []
                for s in range(SL):
                    for hh in range(2):
                        i = 2 * s + hh
                        rlo, rhi = hh * 64, hh * 64 + 64
                        h = 2 * pair + hh
                        pss = pss_pool.tile([P, 1536], f32, tag="pss")
                        pss3 = pss.rearrange("p (b x) -> p b x", x=512)
                        for kc, (kof, ksz) in enumerate(KCH):
                            nc.tensor.matmul(
                                pss3[0:ksz, kc, 0:NP],
                                lhsT=kT[rlo:rhi,
                                        s * NP + kof: s * NP + kof + ksz],
                                rhs=qT[rlo:rhi, s * NP:(s + 1) * NP],
                                start=True, stop=True)
                        et = e_pool.tile([P, 3, NP], f32r, tag="et")
                        nc.scalar.activation(et, pss3[:, :, 0:NP],
                                             AF.Exp, scale=HD ** -0.5)
                        for kc, (kof, ksz) in enumerate(KCH):
                            nc.tensor.matmul(
                                pss[0:65, 0:NP],
                                lhsT=vnat[(s, kc)][:, h, :],
                                rhs=et[0:ksz, kc, :],
                                start=(kc == 0), stop=(kc == 2))
                        nc.vector.tensor_copy(dg[64:65, i], pss[64:65, 0:NP])
                        osl = oall[rlo:rhi, pair, s * N:(s + 1) * N]
                        nc.vector.tensor_copy(osl, pss[0:64, 0:N])
                        osls.append((i, osl))
                        if half == 0 and pair == 0 and i == 0:
                            tap("t_et", et)
                nc.vector.reciprocal(dg[64:65], dg[64:65])
                if half == 0 and pair == 0:
                    tap("t_dgr", dg[64:65])
                # partition_broadcast only works from partition 0 — hop the
                # reciprocal row down via an SBUF->SBUF DMA first.
                dg0 = dg_pool.tile([1, 2 * SL, NP], f32, tag="dg0")
                nc.sync.dma_start(out=dg0, in_=dg[64:65])
                for i, osl in osls:
                    hh = i % 2
                    rlo = hh * 64
                    rb = recb_pool.tile([P, NP], f32, tag="recb")
                    nc.gpsimd.partition_broadcast(rb, dg0[0:1, i])
                    nc.gpsimd.tensor_mul(osl, osl, rb[rlo:rlo + 64, 0:N])

            # ---------------- proj / LN2 ----------------
            def proj_mm(half, oall, wp_eff, bp_row, x2all, psp_pool, pr_pool):
                for i, (of, sz) in enumerate(TCH):
                    for ch in range(2):
                        ca, cw = ch * 384, 384
                        pp = psp_pool.tile([P, 384], f32, tag="psp")
                        for dk in range(6):
                            nc.tensor.matmul(
                                pp[0:sz],
                                lhsT=oall[:, dk, of:of + sz],
                                rhs=wp_eff[:, dk, ca:ca + cw],
                                start=(dk == 0), stop=False)
                        nc.tensor.matmul(
                            pp[0:sz], lhsT=ones_row[:, 0:sz],
                            rhs=bp_row[:, ca:ca + cw],
                            start=False, stop=True)
                        x_t = pr_pool.tile([P, C], f32, tag="resx")
                        nc.sync.dma_start(
                            out=x_t[0:sz, ca:ca + cw],
                            in_=x_flat[half * T + of:half * T + of + sz,
                                       ca:ca + cw])
                        nc.vector.tensor_add(x2all[0:sz, i, ca:ca + cw],
                                             pp[0:sz], x_t[0:sz, ca:ca + cw])

            def ln2_phase(half, x2all, xn2T, ln_pool, pst_pool):
                gc = lncols["n2l_g" if half == 0 else "n2s_g"]
                bc = lncols["n2l_b" if half == 0 else "n2s_b"]
                for i, (of, sz) in enumerate(TCH):
                    x2_t = x2all[0:sz, i]
                    xg2 = x2_t.rearrange("p (g d) -> p g d", g=3)
                    stats = ln_pool.tile([P, 3, 6], f32, tag="ln2_stats")
                    for gi in range(3):
                        nc.vector.bn_stats(out=stats[0:sz, gi], in_=xg2[:, gi])
                    mv = ln_pool.tile([P, 2], f32, tag="ln2_mv")
                    nc.vector.bn_aggr(out=mv[0:sz], in_=stats[0:sz])
                    rstd = ln_pool.tile([P, 1], f32, tag="ln2_rstd")
                    nc.scalar.activation(rstd[0:sz], mv[0:sz, 1:2], AF.Sqrt,
                                         bias=eps_t[0:sz], scale=1.0)
                    nc.vector.reciprocal(rstd[0:sz], rstd[0:sz])
                    pre = ln_pool.tile([P, C], f32r, tag="ln2_pre")
                    nc.vector.tensor_scalar(pre[0:sz], x2_t,
                                            scalar1=mv[0:sz, 0:1],
                                            scalar2=rstd[0:sz],
                                            op0=OP.subtract, op1=OP.mult)
                    for j in range(6):
                        pst = pst_pool.tile([P, P], f32r, tag="pst2")
                        nc.tensor.transpose(pst[:, 0:sz],
                                            pre[0:sz, j * P:(j + 1) * P],
                                            ident_b[0:sz, 0:sz])
                        nc.scalar.activation(xn2T[:, j, of:of + sz],
                                             pst[:, 0:sz], AF.Identity,
                                             bias=bc[:, j:j + 1],
                                             scale=gc[:, j:j + 1])

            # ---------------- MLP ----------------
            def mlp_half(half, xn2T, x2all, w1_sb, w2_sb, b2_eff,
                         h_pool, hs_pool, mo_pool, psf_pool, pso_pool):
                # fold fc2 bias + residual base together (on GpSimd; SBUF only)
                for i, (of, sz) in enumerate(TCH):
                    nc.gpsimd.tensor_add(x2all[0:sz, i], x2all[0:sz, i],
                                         b2_eff[0:sz])
                for (gof, chunks) in GRPS:
                    gsz = sum(s for _, s in chunks)
                    pso = pso_pool.tile([P, 2560], f32, tag="pso")
                    for m in range(24):
                        msl = slice(m * P, (m + 1) * P)
                        if half == 0:
                            pf = psf_pool.tile([P, 512], f32, tag="pf0")
                            for kk in range(6):
                                nc.tensor.matmul(
                                    pf[:, 0:gsz],
                                    lhsT=w1_sb[:, kk, msl],
                                    rhs=xn2T[:, kk, gof:gof + gsz],
                                    start=(kk == 0), stop=(kk == 5))
                            hrow = h_pool.tile([P, 384], f32r, tag="hrow")
                            nc.scalar.activation(hrow[:, 0:gsz], pf[:, 0:gsz],
                                                 AF.Gelu,
                                                 bias=b1_col[:, m:m + 1],
                                                 scale=1.0)
                            for i, (tco, tcs) in enumerate(chunks):
                                hsl = hrow[:, tco:tco + tcs]
                                nc.tensor.matmul(
                                    pso[0:tcs, i * 512:i * 512 + 512],
                                    lhsT=hsl, rhs=w2_sb[:, m, 0:512],
                                    start=(m == 0), stop=(m == 23),
                                    skip_group_check=True)
                                nc.tensor.matmul(
                                    pso[0:tcs, 1536 + i * 256:1792 + i * 256],
                                    lhsT=hsl, rhs=w2_sb[:, m, 512:768],
                                    start=(m == 0 and i != 1),
                                    stop=(m == 23),
                                    skip_group_check=True)
                        else:
                            pf = psf_pool.tile([P, 1024], f32, tag="pf1")
                            # bank0: prefix(c 0:256) = kk 0-1
                            # bank1: prefix(c 0:384) = kk 0-2, then continues
                            #        to the full sum after the h1 snapshot.
                            for kk in range(2):
                                nc.tensor.matmul(
                                    pf[:, 0:gsz], lhsT=w1_sb[:, kk, msl],
                                    rhs=xn2T[:, kk, gof:gof + gsz],
                                    start=(kk == 0), stop=(kk == 1))
                            for kk in range(3):
                                nc.tensor.matmul(
                                    pf[:, 512:512 + gsz],
                                    lhsT=w1_sb[:, kk, msl],
                                    rhs=xn2T[:, kk, gof:gof + gsz],
                                    start=(kk == 0), stop=(kk == 2))
                            h2t = hs_pool.tile([P, 384], f32r, tag="h2t")
                            nc.scalar.activation(h2t[:, 0:gsz], pf[:, 0:gsz],
                                                 AF.Gelu,
                                                 bias=b1_col[:, m:m + 1],
                                                 scale=1.0)
                            h1t = hs_pool.tile([P, 384], f32r, tag="h1t")
                            nc.scalar.activation(h1t[:, 0:gsz],
                                                 pf[:, 512:512 + gsz],
                                                 AF.Gelu,
                                                 bias=b1_col[:, m:m + 1],
                                                 scale=1.0)
                            for kk in range(3, 6):
                                nc.tensor.matmul(
                                    pf[:, 512:512 + gsz],
                                    lhsT=w1_sb[:, kk, msl],
                                    rhs=xn2T[:, kk, gof:gof + gsz],
                                    start=False, stop=(kk == 5),
                                    skip_group_check=True)
                            h0t = hs_pool.tile([P, 384], f32r, tag="h0t")
                            nc.scalar.activation(h0t[:, 0:gsz],
                                                 pf[:, 512:512 + gsz],
                                                 AF.Gelu,
                                                 bias=b1_col[:, m:m + 1],
                                                 scale=1.0)
                            x0 = hs_pool.tile([P, 384], f32r, tag="x0")
                            nc.vector.tensor_scalar_mul(x0[:, 0:gsz],
                                                        h0t[:, 0:gsz], g0c)
                            HBt = hs_pool.tile([P, 384], f32r, tag="HBt")
                            nc.vector.scalar_tensor_tensor(
                                HBt[:, 0:gsz], in0=h1t[:, 0:gsz], scalar=g1c,
                                in1=x0[:, 0:gsz], op0=OP.mult, op1=OP.add)
                            HAt = hs_pool.tile([P, 384], f32r, tag="HAt")
                            nc.vector.scalar_tensor_tensor(
                                HAt[:, 0:gsz], in0=h2t[:, 0:gsz], scalar=g2c,
                                in1=HBt[:, 0:gsz], op0=OP.mult, op1=OP.add)
                            for i, (tco, tcs) in enumerate(chunks):
                                tsl = slice(tco, tco + tcs)
                                for src, ca, cw, st in (
                                        (HAt, 0, 256, m == 0),
                                        (HBt, 256, 128, False),
                                        (x0, 384, 128, False)):
                                    nc.tensor.matmul(
                                        pso[0:tcs,
                                            i * 512 + ca:i * 512 + ca + cw],
                                        lhsT=src[:, tsl],
                                        rhs=w2_sb[:, m, ca:ca + cw],
                                        start=st, stop=(m == 23),
                                        skip_group_check=True)
                                nc.tensor.matmul(
                                    pso[0:tcs, 1536 + i * 256:1792 + i * 256],
                                    lhsT=x0[:, tsl],
                                    rhs=w2_sb[:, m, 512:768],
                                    start=(m == 0 and i != 1),
                                    stop=(m == 23),
                                    skip_group_check=True)
                    for i, (tco, tcs) in enumerate(chunks):
                        of = gof + tco
                        ti = of // P
                        ot = mo_pool.tile([P, C], f32, tag="mo_out")
                        nc.vector.tensor_add(ot[0:tcs, 0:512],
                                             pso[0:tcs, i * 512:i * 512 + 512],
                                             x2all[0:tcs, ti, 0:512])
                        nc.vector.tensor_add(
                            ot[0:tcs, 512:768],
                            pso[0:tcs, 1536 + i * 256:1792 + i * 256],
                            x2all[0:tcs, ti, 512:768])
                        nc.sync.dma_start(
                            out=out_flat[half * T + of:half * T + of + tcs],
                            in_=ot[0:tcs])

            # ================ kernel body ================
            with tc.tile_pool(name="wqkv", bufs=1) as wqkv_pool:
                with tc.tile_pool(name="xn2sb", bufs=1) as xn2_pool, \
                     tc.tile_pool(name="x2sb", bufs=1) as x2_pool:
                    xn2Ts = {h: xn2_pool.tile([P, 6, T], f32r, tag=f"xn2T{h}",
                                              name=f"xn2T{h}") for h in (0, 1)}
                    x2alls = {h: x2_pool.tile([P, len(TCH), C], f32,
                                              tag=f"x2all{h}", name=f"x2all{h}")
                              for h in (0, 1)}

                    with tc.tile_pool(name="oall", bufs=1) as oall_pool:
                        oalls = {h: oall_pool.tile([P, 6, T], f32r,
                                                   tag=f"oall{h}",
                                                   name=f"oall{h}")
                                 for h in (0, 1)}
                        with tc.tile_pool(name="xn1", bufs=1) as xn_pool:
                            xnTs = {h: xn_pool.tile([P, 6, QTL], f32r,
                                                    tag=f"xnT{h}",
                                                    name=f"xnT{h}")
                                    for h in (0, 1)}
                            wqk_sb = xn_pool.tile([P, 6, 2 * C], f32r,
                                                  tag="wqk")
                            nc.gpsimd.dma_start(
                                out=wqk_sb,
                                in_=wqkv_d[:, 0:2 * C].rearrange(
                                    "(j p) n -> p j n", p=P))
                            wv_sb = xn_pool.tile([P, 6, C], f32r, tag="wv")
                            nc.gpsimd.dma_start(
                                out=wv_sb,
                                in_=wqkv_d[:, 2 * C:3 * C].rearrange(
                                    "(j p) n -> p j n", p=P))
                            with tc.tile_pool(name="ln1", bufs=3) as ln_pool, \
                                 tc.tile_pool(name="ps_t1", bufs=2,
                                              space="PSUM") as pst_pool:
                                for half in (0, 1):
                                    ln1_phase(half, xnTs[half], ln_pool,
                                              pst_pool)
                            with tc.tile_pool(name="vnat", bufs=1) as vnat_pool, \
                                 tc.tile_pool(name="qk", bufs=2) as qk_pool, \
                                 tc.tile_pool(name="epool", bufs=2) as e_pool, \
                                 tc.tile_pool(name="dg", bufs=1) as dg_pool, \
                                 tc.tile_pool(name="recb", bufs=4) as recb_pool, \
                                 tc.tile_pool(name="ps_s", bufs=2,
                                              space="PSUM") as pss_pool, \
                                 tc.tile_pool(name="ps_q", bufs=1,
                                              space="PSUM") as pq_pool:
                                for half in (0, 1):
                                    vnat = {}
                                    v_gen(half, xnTs[half], wv_sb, vnat_pool,
                                          pq_pool, vnat)
                                    qts = {}
                                    for pair in range(6):
                                        qt = qk_pool.tile(
                                            [P, QTL], f32r, tag="qT",
                                            name=f"qT{half}_{pair}")
                                        kt = qk_pool.tile(
                                            [P, QTL], f32r, tag="kT",
                                            name=f"kT{half}_{pair}")
                                        qts[pair] = (qt, kt)
                                        qk_gen(half, pair, xnTs[half], wqk_sb,
                                               qts[pair][0], qts[pair][1],
                                               pq_pool)
                                        if half == 0 and pair == 0:
                                            tap("t_q", qt)
                                            tap("t_k", kt)
                                        if pair > 0:
                                            attn_pair(half, pair - 1,
                                                      *qts.pop(pair - 1),
                                                      vnat, oalls[half],
                                                      pss_pool, e_pool,
                                                      dg_pool, recb_pool)
                                    attn_pair(half, 5, *qts.pop(5), vnat,
                                              oalls[half], pss_pool, e_pool,
                                              dg_pool, recb_pool)
                                    if half == 0:
                                        tap("t_xnT", xnTs[0])
                                        tap("t_vn0", vnat[(0, 0)])
                                        tap("t_vnt", vnat[(0, 2)])
                                        tap("t_oall", oalls[0])

                        # ---- proj (attention pools closed; xn1 still open
                        # but unused past here — closed by scope above) ----
                        with tc.tile_pool(name="wp", bufs=1) as wp_pool, \
                             tc.tile_pool(name="prtmp", bufs=3) as pr_pool, \
                             tc.tile_pool(name="ps_p", bufs=4,
                                          space="PSUM") as psp_pool:
                            wproj_sb = wp_pool.tile([P, 6, C], f32r, tag="wproj")
                            nc.gpsimd.dma_start(
                                out=wproj_sb,
                                in_=wproj_d.rearrange("(j p) n -> p j n", p=P))
                            bp_b = wp_pool.tile([P, C], f32, tag="bp_b")
                            nc.gpsimd.dma_start(out=bp_b,
                                                in_=bcast_row(vecs["b_proj"]))
                            bp_row = wp_pool.tile([1, C], f32r, tag="bp_row")
                            nc.vector.tensor_copy(bp_row, bp_b[0:1])
                            wt_sb = wp_pool.tile([P, 6, C], f32r, tag="wtilde")
                            for j in range(6):
                                if j < 2:
                                    small_scale3(wt_sb[:, j], wproj_sb[:, j],
                                                 [((0, 256), g012c),
                                                  ((256, 384), g01c),
                                                  ((384, 768), g0c)])
                                elif j == 2:
                                    small_scale3(wt_sb[:, j], wproj_sb[:, j],
                                                 [((0, 384), g01c),
                                                  ((384, 768), g0c)])
                                else:
                                    small_scale3(wt_sb[:, j], wproj_sb[:, j],
                                                 [((0, 768), g0c)])
                            bt_b = wp_pool.tile([P, C], f32, tag="btilde")
                            small_scale3(bt_b, bp_b,
                                         [((0, 256), g012c), ((256, 384), g01c),
                                          ((384, 768), g0c)])
                            bt_row = wp_pool.tile([1, C], f32r, tag="bt_row")
                            nc.vector.tensor_copy(bt_row, bt_b[0:1])
                            for half in (0, 1):
                                proj_mm(half, oalls[half],
                                        wproj_sb if half == 0 else wt_sb,
                                        bp_row if half == 0 else bt_row,
                                        x2alls[half], psp_pool, pr_pool)
                            tap("t_x2", x2alls[0])

                    # ---- LN2 + MLP (oall closed; w1/w2 load overlaps) ----
                    with tc.tile_pool(name="wmlp", bufs=1) as wm_pool:
                        w1_sb = wm_pool.tile([P, 6, HID], f32r, tag="w1")
                        w2_sb = wm_pool.tile([P, 24, C], f32r, tag="w2")
                        w1_r = wfc1_d.rearrange("(j p) n -> p j n", p=P)
                        w2_r = wfc2_d.rearrange("(j p) n -> p j n", p=P)
                        for q in range(4):
                            nc.gpsimd.dma_start(
                                out=w1_sb[:, :, q * 768:(q + 1) * 768],
                                in_=w1_r[:, :, q * 768:(q + 1) * 768])
                            nc.gpsimd.dma_start(
                                out=w2_sb[:, q * 6:(q + 1) * 6],
                                in_=w2_r[:, q * 6:(q + 1) * 6])
                        b2_b = wm_pool.tile([P, C], f32, tag="b2_b")
                        nc.gpsimd.dma_start(out=b2_b,
                                            in_=bcast_row(vecs["b_fc2"]))
                        b2t_b = wm_pool.tile([P, C], f32, tag="b2t_b")
                        small_scale3(b2t_b, b2_b,
                                     [((0, 256), g012c), ((256, 384), g01c),
                                      ((384, 768), g0c)])

                        with tc.tile_pool(name="ln2", bufs=3) as ln2_pool, \
                             tc.tile_pool(name="ps_t2", bufs=2,
                                          space="PSUM") as pst2_pool:
                            for half in (0, 1):
                                ln2_phase(half, x2alls[half], xn2Ts[half],
                                          ln2_pool, pst2_pool)
                            tap("t_xn2", xn2Ts[0])

                        with tc.tile_pool(name="hrows", bufs=2) as h_pool, \
                             tc.tile_pool(name="hsm", bufs=2) as hs_pool, \
                             tc.tile_pool(name="mout", bufs=3) as mo_pool, \
                             tc.tile_pool(name="ps_out", bufs=1,
                                          space="PSUM") as pso_pool:
                            with tc.tile_pool(name="ps_f0", bufs=2,
                                              space="PSUM") as psf0_pool:
                                mlp_half(0, xn2Ts[0], x2alls[0], w1_sb, w2_sb,
                                         b2_b, h_pool, hs_pool, mo_pool,
                                         psf0_pool, pso_pool)
                            with tc.tile_pool(name="ps_f1", bufs=1,
                                              space="PSUM") as psf1_pool:
                                mlp_half(1, xn2Ts[1], x2alls[1], w1_sb, w2_sb,
                                         b2t_b, h_pool, hs_pool, mo_pool,
                                         psf1_pool, pso_pool)

    nc.compile()
    nc._tap_list = tap_list
    return nc


def _get_nc():
    if "nc" not in _CACHE:
        _CACHE["nc"] = _build()
    return _CACHE["nc"]


def kernel(**inputs):
    from concourse import bass_utils

    nc = _get_nc()
    arrs = {k: np.ascontiguousarray(np.asarray(v, dtype=np.float32))
            for k, v in inputs.items()}
    x = arrs.pop("x")
    B = x.shape[0]
    B2 = B // 2
    per = B2 // NCORES
    in_maps = []
    for c in range(NCORES):
        shard = np.concatenate([x[c * per:(c + 1) * per],
                                x[B2 + c * per:B2 + (c + 1) * per]], axis=0)
        m = {"x": np.ascontiguousarray(shard)}
        m.update(arrs)
        in_maps.append(m)
    res = bass_utils.run_bass_kernel_spmd(nc, in_maps,
                                          core_ids=list(range(NCORES)))
    out = np.empty_like(x.reshape(B, N, C))
    for c in range(NCORES):
        o = res.results[c]["out"]
        out[c * per:(c + 1) * per] = o[:per]
        out[B2 + c * per:B2 + (c + 1) * per] = o[per:]
    return out
